# revision 8
# baseline (speedup 1.0000x reference)
"""Masked multi-head attention kernel for 8 Trainium2 NeuronCores.

Strategy:
  - 24 (batch, head) pairs sharded as: core c -> batch c//4, heads [3*(c%4) .. 3*(c%4)+2].
  - Key-padding mask handled by HOST-side gather: only unmasked key positions are
    shipped/computed (mask kills ~50% of keys -> scores/exp/AV work halves).
    Padded key slots get zeroed K columns (scores=0 -> exp=1) and a 0 in the
    indicator column of V, so they contribute nothing to numerator or denominator.
  - Softmax without max-subtraction (scores ~ N(0,1); masked keys excluded, exact
    match: reference's masked exp underflows to 0 in fp32).
  - Row-sum of exp folded into the AV matmul via an extra indicator column on V.
  - bf16 matmul inputs, fp32 PSUM accumulation.
  - Per-core output = transposed partial projection [768, 2048]; host sums the 4
    partials per batch, transposes, adds proj_b.
Engine split: PE matmuls; ACT only exp; DVE copies/recip/mul; DMA via sync.
"""

import math

import numpy as np
import ml_dtypes

BF16 = ml_dtypes.bfloat16
B, N, C = 2, 2048, 768
H = 12
D = 64
HPC = 3          # heads per core
P = 128
SCALE = D ** -0.5
NCORES = 8


def _build_program(KP: int):
    from concourse import bacc, mybir
    from concourse.tile import TileContext

    JG = KP // P
    f32 = mybir.dt.float32
    bf16 = mybir.dt.bfloat16
    nc = bacc.Bacc(None, target_bir_lowering=False)

    xT_d = nc.declare_dram_parameter("xT", [P, 6, N], bf16, False)
    xTk_d = nc.declare_dram_parameter("xTk", [P, 6, KP], bf16, False)
    kf_d = nc.declare_dram_parameter("kf", [P, JG], bf16, False)
    wq_d = nc.declare_dram_parameter("wqT", [P, 6, 192], bf16, False)
    wk_d = nc.declare_dram_parameter("wkT", [P, 6, 192], bf16, False)
    wv_d = nc.declare_dram_parameter("wvT", [P, 6, 192], bf16, False)
    pT_d = nc.declare_dram_parameter("pT", [D, HPC, C], bf16, False)
    out_d = nc.declare_dram_parameter("outT", [P, 6, N], f32, True)

    with TileContext(nc) as tc:
        with (
            tc.tile_pool(name="const", bufs=1) as cpool,
            tc.tile_pool(name="work", bufs=1) as wpool,
            tc.tile_pool(name="pt", bufs=3) as ptpool,
            tc.tile_pool(name="rrow", bufs=2) as rpool,
            tc.tile_pool(name="rb", bufs=4) as rbpool,
            tc.tile_pool(name="outp", bufs=3) as opool,
            tc.tile_pool(name="ps", bufs=2, space="PSUM") as pspool,
            tc.tile_pool(name="po", bufs=1, space="PSUM") as popool,
        ):
            # ---- load inputs
            xT = cpool.tile([P, 6, N], bf16)
            nc.sync.dma_start(xT[:], xT_d[:])
            xTk = cpool.tile([P, 6, KP], bf16)
            nc.sync.dma_start(xTk[:], xTk_d[:])
            kf = cpool.tile([P, JG], bf16)
            nc.sync.dma_start(kf[:], kf_d[:])
            wq = cpool.tile([P, 6, 192], bf16)
            nc.sync.dma_start(wq[:], wq_d[:])
            wk = cpool.tile([P, 6, 192], bf16)
            nc.sync.dma_start(wk[:], wk_d[:])
            wv = cpool.tile([P, 6, 192], bf16)
            nc.sync.dma_start(wv[:], wv_d[:])
            pT = cpool.tile([D, HPC, C], bf16)
            nc.sync.dma_start(pT[:], pT_d[:])
            ones = cpool.tile([1, D], f32)
            nc.vector.memset(ones[:], 1.0)

            # ---- qT / kT projections (2 heads packed per 128-row group)
            qT01 = wpool.tile([P, N], bf16)
            qT2 = wpool.tile([D, N], bf16)
            kT01 = wpool.tile([P, KP], bf16)
            kT2 = wpool.tile([D, KP], bf16)

            for (w_sb, dst_list, ncols) in (
                (wq, [(0, P, qT01), (P, D, qT2)], N),
                (wk, [(0, P, kT01), (P, D, kT2)], KP),
            ):
                for (m0, msz, dst) in dst_list:
                    for n0 in range(0, ncols, 1024):
                        nsz = min(1024, ncols - n0)
                        pq = pspool.tile([P, 1024], f32, tag="ps")
                        for s0 in range(0, nsz, 512):
                            ssz = min(512, nsz - s0)
                            for t in range(6):
                                nc.tensor.matmul(
                                    pq[:msz, s0 : s0 + ssz],
                                    w_sb[:, t, m0 : m0 + msz],
                                    (xT if ncols == N else xTk)[:, t, n0 + s0 : n0 + s0 + ssz],
                                    start=(t == 0),
                                    stop=(t == 5),
                                )
                        nc.vector.tensor_copy(dst[:msz, n0 : n0 + nsz], pq[:msz, :nsz])

            # ---- V in [key, dim] layout with indicator column (direct matmul)
            v_sb = wpool.tile([P, JG, HPC, D + 1], bf16)
            for jg in range(JG):
                pv = pspool.tile([P, 1024], f32, tag="ps")
                for t in range(6):
                    nc.tensor.matmul(
                        pv[:, :192],
                        xTk[:, t, jg * P : (jg + 1) * P],
                        wv[:, t, :],
                        start=(t == 0),
                        stop=(t == 5),
                    )
                for ih in range(HPC):
                    nc.vector.tensor_copy(
                        v_sb[:, jg, ih, 0:D], pv[:, ih * D : (ih + 1) * D]
                    )
                    nc.vector.tensor_copy(v_sb[:, jg, ih, D : D + 1], kf[:, jg : jg + 1])

            # ---- attention per head
            OT = wpool.tile([D, HPC, N], bf16)
            for ih in range(HPC):
                qT_h = qT01[D * ih : D * (ih + 1), :] if ih < 2 else qT2[:, :]
                kT_h = kT01[D * ih : D * (ih + 1), :] if ih < 2 else kT2[:, :]
                po_t = popool.tile([D + 1, N], f32)
                for jg in range(JG):
                    pt_t = ptpool.tile([P, N], bf16)
                    for n0 in range(0, N, 1024):
                        psc = pspool.tile([P, 1024], f32, tag="ps")
                        for s0 in (0, 512):
                            nc.tensor.matmul(
                                psc[:, s0 : s0 + 512],
                                kT_h[:, jg * P : (jg + 1) * P],
                                qT_h[:, n0 + s0 : n0 + s0 + 512],
                                start=True,
                                stop=True,
                            )
                        nc.scalar.activation(
                            pt_t[:, n0 : n0 + 1024],
                            psc[:],
                            mybir.ActivationFunctionType.Exp,
                            scale=float(SCALE),
                        )
                    for n0 in range(0, N, 512):
                        nc.tensor.matmul(
                            po_t[:, n0 : n0 + 512],
                            v_sb[:, jg, ih, :],
                            pt_t[:, n0 : n0 + 512],
                            start=(jg == 0),
                            stop=(jg == JG - 1),
                        )
                # normalize: copy row-sum (partition D of po_t) to SBUF, broadcast
                # the SUM across 64 partitions via K=1 matmul, then fast reciprocal
                # and multiply.
                r_row = rpool.tile([1, N], f32, tag="rrow")
                nc.vector.tensor_copy(r_row[:], po_t[D : D + 1, :])
                for n0 in range(0, N, 512):
                    pb = pspool.tile([P, 1024], f32, tag="ps")
                    nc.tensor.matmul(
                        pb[:D, :512], ones[:], r_row[:, n0 : n0 + 512],
                        start=True, stop=True,
                    )
                    rb_t = rbpool.tile([D, 512], f32, tag="rb")
                    nc.vector.reciprocal_approx_fast(rb_t[:], pb[:D, :512])
                    nc.vector.tensor_mul(
                        OT[:, ih, n0 : n0 + 512],
                        po_t[0:D, n0 : n0 + 512],
                        rb_t[:],
                    )

            # ---- partial projection, transposed output
            for cg in range(6):
                for n0 in range(0, N, 1024):
                    pp = pspool.tile([P, 1024], f32, tag="ps")
                    for s0 in (0, 512):
                        for ih in range(HPC):
                            nc.tensor.matmul(
                                pp[:, s0 : s0 + 512],
                                pT[:, ih, cg * P : (cg + 1) * P],
                                OT[:, ih, n0 + s0 : n0 + s0 + 512],
                                start=(ih == 0),
                                stop=(ih == HPC - 1),
                            )
                    ob = opool.tile([P, 1024], f32)
                    nc.vector.tensor_copy(ob[:], pp[:])
                    nc.sync.dma_start(out_d[:, cg, n0 : n0 + 1024], ob[:])

    nc.finalize()
    return nc


def _prep_inputs(x, mask, qkv_w, proj_w):
    """Build the 8 per-core input maps. Returns (in_maps, KP)."""
    idx = [np.nonzero(mask[b] == 0.0)[0] for b in range(B)]
    nk = max(len(i) for i in idx)
    KP = max(P, int(math.ceil(nk / P)) * P)
    JG = KP // P

    per_batch = []
    for b in range(B):
        xTb = np.ascontiguousarray(x[b].T)  # [C, N] f32
        xT_in = xTb.reshape(6, P, N).transpose(1, 0, 2).astype(BF16)
        xk = np.zeros((C, KP), np.float32)
        xk[:, : len(idx[b])] = xTb[:, idx[b]]
        xTk_in = xk.reshape(6, P, KP).transpose(1, 0, 2).astype(BF16)
        kfv = np.zeros((KP,), np.float32)
        kfv[: len(idx[b])] = 1.0
        kf_in = kfv.reshape(JG, P).T.astype(BF16)
        per_batch.append((xT_in, xTk_in, kf_in))

    in_maps = []
    for c in range(NCORES):
        b, g = c // 4, c % 4
        h0 = HPC * g
        xT_in, xTk_in, kf_in = per_batch[b]
        m = {"xT": xT_in, "xTk": xTk_in, "kf": kf_in}
        for name, off in (("wqT", 0), ("wkT", C), ("wvT", 2 * C)):
            w = qkv_w[off + h0 * D : off + (h0 + HPC) * D]  # [192, C]
            m[name] = (
                np.ascontiguousarray(w.T).reshape(6, P, 192).transpose(1, 0, 2).astype(BF16)
            )
        pT = np.stack(
            [proj_w[:, (h0 + ih) * D : (h0 + ih + 1) * D].T for ih in range(HPC)],
            axis=1,
        )  # [64, 3, 768]
        m["pT"] = pT.astype(BF16)
        in_maps.append(m)
    return in_maps, KP


_CACHE = {}


def _get_program(KP):
    if KP not in _CACHE:
        _CACHE[KP] = _build_program(KP)
    return _CACHE[KP]


def kernel(x, mask, qkv_w, proj_w, proj_b, _want_results=False):
    from concourse.bass_utils import run_bass_kernel_spmd

    x = np.asarray(x, np.float32)
    mask = np.asarray(mask, np.float32)
    qkv_w = np.asarray(qkv_w, np.float32)
    proj_w = np.asarray(proj_w, np.float32)
    proj_b = np.asarray(proj_b, np.float32)

    in_maps, KP = _prep_inputs(x, mask, qkv_w, proj_w)
    nc = _get_program(KP)
    res = run_bass_kernel_spmd(nc, in_maps, list(range(NCORES)))

    out = np.empty((B, N, C), np.float32)
    for b in range(B):
        acc = None
        for c in range(4 * b, 4 * b + 4):
            a = res.results[c]["outT"]  # [128, 6, N]
            a = np.asarray(a, np.float32).transpose(1, 0, 2).reshape(C, N)
            acc = a if acc is None else acc + a
        out[b] = acc.T + proj_b[None, :]
    if _want_results:
        return out, res
    return out


# revision 20
# speedup vs baseline: 1.2748x; 1.2748x over previous
"""Masked multi-head attention kernel for 8 Trainium2 NeuronCores.

Strategy:
  - 24 (batch, head) pairs sharded as: core c -> batch c//4, heads [3*(c%4) .. 3*(c%4)+2].
  - Key-padding mask handled by HOST-side gather: only unmasked key positions are
    shipped/computed (mask kills ~50% of keys -> scores/exp/AV work halves).
    Padded key slots get zeroed K columns (scores=0 -> exp=1) and a 0 in the
    indicator column of V, so they contribute nothing to numerator or denominator.
  - Softmax without max-subtraction (scores ~ N(0,1); masked keys excluded, exact
    match: reference's masked exp underflows to 0 in fp32).
  - Row-sum of exp folded into the AV matmul via an extra indicator column on V.
  - bf16 matmul inputs, fp32 PSUM accumulation.
  - Per-core output = transposed partial projection [768, 2048]; host sums the 4
    partials per batch, transposes, adds proj_b.
Engine split: PE matmuls; ACT only exp; DVE copies/recip/mul; DMA via sync.
"""

import math

import numpy as np
import ml_dtypes

BF16 = ml_dtypes.bfloat16
B, N, C = 2, 2048, 768
H = 12
D = 64
HPC = 3          # heads per core
P = 128
SCALE = D ** -0.5
NCORES = 8


def _build_program(KP: int):
    from concourse import bacc, mybir
    from concourse.tile import TileContext

    JG = KP // P
    f32 = mybir.dt.float32
    bf16 = mybir.dt.bfloat16
    nc = bacc.Bacc(None, target_bir_lowering=False)

    xT_d = nc.declare_dram_parameter("xT", [P, 6, N], bf16, False)
    xTk_d = nc.declare_dram_parameter("xTk", [P, 6, KP], bf16, False)
    kf_d = nc.declare_dram_parameter("kf", [P, JG, D], bf16, False)
    wq_d = nc.declare_dram_parameter("wqT", [P, 6, 192], bf16, False)
    wk_d = nc.declare_dram_parameter("wkT", [P, 6, 192], bf16, False)
    wv_d = nc.declare_dram_parameter("wvT", [P, 6, 192], bf16, False)
    pT_d = nc.declare_dram_parameter("pT", [D, HPC, C], bf16, False)
    out_d = nc.declare_dram_parameter("outT", [P, 6, N], f32, True)

    with TileContext(nc) as tc:
        with (
            tc.tile_pool(name="const", bufs=1) as cpool,
            tc.tile_pool(name="work", bufs=1) as wpool,
            tc.tile_pool(name="pt", bufs=3) as ptpool,
            tc.tile_pool(name="rb", bufs=4) as rbpool,
            tc.tile_pool(name="outp", bufs=3) as opool,
            tc.tile_pool(name="ps", bufs=2, space="PSUM") as pspool,
            tc.tile_pool(name="po", bufs=1, space="PSUM") as popool,
        ):
            # ---- load inputs
            xT = cpool.tile([P, 6, N], bf16)
            nc.sync.dma_start(xT[:], xT_d[:])
            xTk = cpool.tile([P, 6, KP], bf16)
            nc.sync.dma_start(xTk[:], xTk_d[:])
            kf = cpool.tile([P, JG, D], bf16)
            nc.sync.dma_start(kf[:], kf_d[:])
            wq = cpool.tile([P, 6, 192], bf16)
            nc.sync.dma_start(wq[:], wq_d[:])
            wk = cpool.tile([P, 6, 192], bf16)
            nc.sync.dma_start(wk[:], wk_d[:])
            wv = cpool.tile([P, 6, 192], bf16)
            nc.sync.dma_start(wv[:], wv_d[:])
            pT = cpool.tile([D, HPC, C], bf16)
            nc.sync.dma_start(pT[:], pT_d[:])

            # ---- qT / kT projections (2 heads packed per 128-row group)
            qT01 = wpool.tile([P, N], bf16)
            qT2 = wpool.tile([D, N], bf16)
            kT01 = wpool.tile([P, KP], bf16)
            kT2 = wpool.tile([D, KP], bf16)

            for (w_sb, dst_list, ncols) in (
                (wq, [(0, P, qT01), (P, D, qT2)], N),
                (wk, [(0, P, kT01), (P, D, kT2)], KP),
            ):
                for (m0, msz, dst) in dst_list:
                    for n0 in range(0, ncols, 1024):
                        nsz = min(1024, ncols - n0)
                        pq = pspool.tile([P, 1024], f32, tag="ps")
                        for s0 in range(0, nsz, 512):
                            ssz = min(512, nsz - s0)
                            for t in range(6):
                                nc.tensor.matmul(
                                    pq[:msz, s0 : s0 + ssz],
                                    w_sb[:, t, m0 : m0 + msz],
                                    (xT if ncols == N else xTk)[:, t, n0 + s0 : n0 + s0 + ssz],
                                    start=(t == 0),
                                    stop=(t == 5),
                                )
                        nc.vector.tensor_copy(dst[:msz, n0 : n0 + nsz], pq[:msz, :nsz])

            # ---- V in [key, dim] layout; cols D..2D-1 hold the keep-indicator
            # replicated 64x so the AV matmul emits the softmax denominator
            # broadcast across 64 partitions for free.
            v_sb = wpool.tile([P, JG, HPC, 2 * D], bf16)
            for jg in range(JG):
                pv = pspool.tile([P, 1024], f32, tag="ps")
                for t in range(6):
                    nc.tensor.matmul(
                        pv[:, :192],
                        xTk[:, t, jg * P : (jg + 1) * P],
                        wv[:, t, :],
                        start=(t == 0),
                        stop=(t == 5),
                    )
                for ih in range(HPC):
                    nc.vector.tensor_copy(v_sb[:, jg, ih, 0:D], kf[:, jg, :])
                    nc.vector.tensor_copy(
                        v_sb[:, jg, ih, D : 2 * D], pv[:, ih * D : (ih + 1) * D]
                    )

            # ---- attention per head
            OT = wpool.tile([D, HPC, N], bf16)
            for ih in range(HPC):
                qT_h = qT01[D * ih : D * (ih + 1), :] if ih < 2 else qT2[:, :]
                kT_h = kT01[D * ih : D * (ih + 1), :] if ih < 2 else kT2[:, :]
                po_t = popool.tile([P, N], f32)
                for jg in range(JG):
                    pt_t = ptpool.tile([P, N], bf16)
                    for n0 in range(0, N, 1024):
                        psc = pspool.tile([P, 1024], f32, tag="ps")
                        for s0 in (0, 512):
                            nc.tensor.matmul(
                                psc[:, s0 : s0 + 512],
                                kT_h[:, jg * P : (jg + 1) * P],
                                qT_h[:, n0 + s0 : n0 + s0 + 512],
                                start=True,
                                stop=True,
                            )
                        nc.scalar.activation(
                            pt_t[:, n0 : n0 + 1024],
                            psc[:],
                            mybir.ActivationFunctionType.Exp,
                            scale=float(SCALE),
                        )
                    for n0 in range(0, N, 512):
                        nc.tensor.matmul(
                            po_t[:, n0 : n0 + 512],
                            v_sb[:, jg, ih, :],
                            pt_t[:, n0 : n0 + 512],
                            start=(jg == 0),
                            stop=(jg == JG - 1),
                        )
                # normalize: partitions 0..D-1 of po_t hold the row-sum broadcast
                # across 64 partitions (indicator cols 0..D-1 of v_sb); partitions
                # D..2D-1 hold O.T. Reciprocal + multiply directly from PSUM.
                for n0 in range(0, N, 512):
                    rb_t = rbpool.tile([D, 512], f32, tag="rb")
                    nc.vector.reciprocal_approx_fast(
                        rb_t[:], po_t[0:D, n0 : n0 + 512]
                    )
                    nc.vector.tensor_mul(
                        OT[:, ih, n0 : n0 + 512],
                        po_t[D : 2 * D, n0 : n0 + 512],
                        rb_t[:],
                    )

            # ---- partial projection, transposed output
            for cg in range(6):
                for n0 in range(0, N, 1024):
                    pp = pspool.tile([P, 1024], f32, tag="ps")
                    for s0 in (0, 512):
                        for ih in range(HPC):
                            nc.tensor.matmul(
                                pp[:, s0 : s0 + 512],
                                pT[:, ih, cg * P : (cg + 1) * P],
                                OT[:, ih, n0 + s0 : n0 + s0 + 512],
                                start=(ih == 0),
                                stop=(ih == HPC - 1),
                            )
                    ob = opool.tile([P, 1024], f32)
                    if cg % 2 == 0:
                        nc.vector.tensor_copy(ob[:], pp[:])
                    else:
                        nc.scalar.copy(ob[:], pp[:])
                    nc.sync.dma_start(out_d[:, cg, n0 : n0 + 1024], ob[:])

    nc.finalize()
    return nc


def _prep_inputs(x, mask, qkv_w, proj_w):
    """Build the 8 per-core input maps. Returns (in_maps, KP)."""
    idx = [np.nonzero(mask[b] == 0.0)[0] for b in range(B)]
    nk = max(len(i) for i in idx)
    KP = max(P, int(math.ceil(nk / P)) * P)
    JG = KP // P

    per_batch = []
    for b in range(B):
        xTb = np.ascontiguousarray(x[b].T)  # [C, N] f32
        xT_in = xTb.reshape(6, P, N).transpose(1, 0, 2).astype(BF16)
        xk = np.zeros((C, KP), np.float32)
        xk[:, : len(idx[b])] = xTb[:, idx[b]]
        xTk_in = xk.reshape(6, P, KP).transpose(1, 0, 2).astype(BF16)
        kfv = np.zeros((KP,), np.float32)
        kfv[: len(idx[b])] = 1.0
        kf_in = np.ascontiguousarray(
            np.broadcast_to(kfv.reshape(JG, P).T[:, :, None], (P, JG, D))
        ).astype(BF16)
        per_batch.append((xT_in, xTk_in, kf_in))

    in_maps = []
    for c in range(NCORES):
        b, g = c // 4, c % 4
        h0 = HPC * g
        xT_in, xTk_in, kf_in = per_batch[b]
        m = {"xT": xT_in, "xTk": xTk_in, "kf": kf_in}
        for name, off in (("wqT", 0), ("wkT", C), ("wvT", 2 * C)):
            w = qkv_w[off + h0 * D : off + (h0 + HPC) * D]  # [192, C]
            m[name] = (
                np.ascontiguousarray(w.T).reshape(6, P, 192).transpose(1, 0, 2).astype(BF16)
            )
        pT = np.stack(
            [proj_w[:, (h0 + ih) * D : (h0 + ih + 1) * D].T for ih in range(HPC)],
            axis=1,
        )  # [64, 3, 768]
        m["pT"] = pT.astype(BF16)
        in_maps.append(m)
    return in_maps, KP


_CACHE = {}


def _get_program(KP):
    if KP not in _CACHE:
        _CACHE[KP] = _build_program(KP)
    return _CACHE[KP]


def kernel(x, mask, qkv_w, proj_w, proj_b, _want_results=False):
    from concourse.bass_utils import run_bass_kernel_spmd

    x = np.asarray(x, np.float32)
    mask = np.asarray(mask, np.float32)
    qkv_w = np.asarray(qkv_w, np.float32)
    proj_w = np.asarray(proj_w, np.float32)
    proj_b = np.asarray(proj_b, np.float32)

    in_maps, KP = _prep_inputs(x, mask, qkv_w, proj_w)
    nc = _get_program(KP)
    res = run_bass_kernel_spmd(nc, in_maps, list(range(NCORES)))

    out = np.empty((B, N, C), np.float32)
    for b in range(B):
        acc = None
        for c in range(4 * b, 4 * b + 4):
            a = res.results[c]["outT"]  # [128, 6, N]
            a = np.asarray(a, np.float32).transpose(1, 0, 2).reshape(C, N)
            acc = a if acc is None else acc + a
        out[b] = acc.T + proj_b[None, :]
    if _want_results:
        return out, res
    return out


# revision 23
# speedup vs baseline: 1.4073x; 1.1039x over previous
"""Masked multi-head attention kernel for 8 Trainium2 NeuronCores.

Strategy:
  - 24 (batch, head) pairs sharded as: core c -> batch c//4, heads [3*(c%4) .. 3*(c%4)+2].
  - Key-padding mask handled by HOST-side gather: only unmasked key positions are
    shipped/computed (mask kills ~50% of keys -> scores/exp/AV work halves).
    Padded key slots get zeroed K columns (scores=0 -> exp=1) and a 0 in the
    indicator column of V, so they contribute nothing to numerator or denominator.
  - Softmax without max-subtraction (scores ~ N(0,1); masked keys excluded, exact
    match: reference's masked exp underflows to 0 in fp32).
  - Row-sum of exp folded into the AV matmul via an extra indicator column on V.
  - bf16 matmul inputs, fp32 PSUM accumulation.
  - Per-core output = transposed partial projection [768, 2048]; host sums the 4
    partials per batch, transposes, adds proj_b.
Engine split: PE matmuls; ACT only exp; DVE copies/recip/mul; DMA via sync.
"""

import math

import numpy as np
import ml_dtypes

BF16 = ml_dtypes.bfloat16
B, N, C = 2, 2048, 768
H = 12
D = 64
HPC = 3          # heads per core
P = 128
SCALE = D ** -0.5
NCORES = 8


def _build_program(KP: int):
    from concourse import bacc, mybir
    from concourse.tile import TileContext

    JG = KP // P
    f32 = mybir.dt.float32
    bf16 = mybir.dt.bfloat16
    nc = bacc.Bacc(None, target_bir_lowering=False)

    xT_d = nc.declare_dram_parameter("xT", [P, 6, N], bf16, False)
    xTk_d = nc.declare_dram_parameter("xTk", [P, 6, KP], bf16, False)
    kf_d = nc.declare_dram_parameter("kf", [P, JG, D], bf16, False)
    wq_d = nc.declare_dram_parameter("wqT", [P, 6, 192], bf16, False)
    wk_d = nc.declare_dram_parameter("wkT", [P, 6, 192], bf16, False)
    wv_d = nc.declare_dram_parameter("wvT", [P, 6, 192], bf16, False)
    pT_d = nc.declare_dram_parameter("pT", [D, HPC, C], bf16, False)
    out_d = nc.declare_dram_parameter("outT", [P, 6, N], f32, True)

    with TileContext(nc) as tc:
        with (
            tc.tile_pool(name="const", bufs=1) as cpool,
            tc.tile_pool(name="work", bufs=1) as wpool,
            tc.tile_pool(name="pt", bufs=3) as ptpool,
            tc.tile_pool(name="rb", bufs=4) as rbpool,
            tc.tile_pool(name="outp", bufs=3) as opool,
            tc.tile_pool(name="ps", bufs=2, space="PSUM") as pspool,
            tc.tile_pool(name="po", bufs=2, space="PSUM") as popool,
        ):
            # ---- load inputs
            xT = cpool.tile([P, 6, N], bf16)
            nc.sync.dma_start(xT[:], xT_d[:])
            xTk = cpool.tile([P, 6, KP], bf16)
            nc.sync.dma_start(xTk[:], xTk_d[:])
            kf = cpool.tile([P, JG, D], bf16)
            nc.sync.dma_start(kf[:], kf_d[:])
            wq = cpool.tile([P, 6, 192], bf16)
            nc.sync.dma_start(wq[:], wq_d[:])
            wk = cpool.tile([P, 6, 192], bf16)
            nc.sync.dma_start(wk[:], wk_d[:])
            wv = cpool.tile([P, 6, 192], bf16)
            nc.sync.dma_start(wv[:], wv_d[:])
            pT = cpool.tile([D, HPC, C], bf16)
            nc.sync.dma_start(pT[:], pT_d[:])

            # ---- qT / kT projections (2 heads packed per 128-row group)
            qT01 = wpool.tile([P, N], bf16)
            qT2 = wpool.tile([D, N], bf16)
            kT01 = wpool.tile([P, KP], bf16)
            kT2 = wpool.tile([D, KP], bf16)

            for (w_sb, dst_list, ncols) in (
                (wq, [(0, P, qT01), (P, D, qT2)], N),
                (wk, [(0, P, kT01), (P, D, kT2)], KP),
            ):
                for (m0, msz, dst) in dst_list:
                    for n0 in range(0, ncols, 1024):
                        nsz = min(1024, ncols - n0)
                        pq = pspool.tile([P, 1024], f32, tag="ps")
                        for s0 in range(0, nsz, 512):
                            ssz = min(512, nsz - s0)
                            for t in range(6):
                                nc.tensor.matmul(
                                    pq[:msz, s0 : s0 + ssz],
                                    w_sb[:, t, m0 : m0 + msz],
                                    (xT if ncols == N else xTk)[:, t, n0 + s0 : n0 + s0 + ssz],
                                    start=(t == 0),
                                    stop=(t == 5),
                                )
                        nc.vector.tensor_copy(dst[:msz, n0 : n0 + nsz], pq[:msz, :nsz])

            # ---- V in [key, dim] layout; cols D..2D-1 hold the keep-indicator
            # replicated 64x so the AV matmul emits the softmax denominator
            # broadcast across 64 partitions for free.
            v_sb = wpool.tile([P, JG, HPC, 2 * D], bf16)
            for jg in range(JG):
                pv = pspool.tile([P, 1024], f32, tag="ps")
                for t in range(6):
                    nc.tensor.matmul(
                        pv[:, :192],
                        xTk[:, t, jg * P : (jg + 1) * P],
                        wv[:, t, :],
                        start=(t == 0),
                        stop=(t == 5),
                    )
                for ih in range(HPC):
                    nc.vector.tensor_copy(v_sb[:, jg, ih, 0:D], kf[:, jg, :])
                    nc.vector.tensor_copy(
                        v_sb[:, jg, ih, D : 2 * D], pv[:, ih * D : (ih + 1) * D]
                    )

            # ---- attention per half-head (1024 queries): two halves in flight
            # (po bufs=2) so PE always has the other half's scores to issue
            # while ScalarE's exp lags.
            NH = N // 2
            OT = wpool.tile([D, HPC, N], bf16)
            for ih in range(HPC):
                qT_h = qT01[D * ih : D * (ih + 1), :] if ih < 2 else qT2[:, :]
                kT_h = kT01[D * ih : D * (ih + 1), :] if ih < 2 else kT2[:, :]
                for nh in range(2):
                    q0 = nh * NH
                    po_t = popool.tile([P, NH], f32)
                    for jg in range(JG):
                        pt_t = ptpool.tile([P, NH], bf16)
                        psc = pspool.tile([P, 1024], f32, tag="ps")
                        for s0 in (0, 512):
                            nc.tensor.matmul(
                                psc[:, s0 : s0 + 512],
                                kT_h[:, jg * P : (jg + 1) * P],
                                qT_h[:, q0 + s0 : q0 + s0 + 512],
                                start=True,
                                stop=True,
                            )
                        nc.scalar.activation(
                            pt_t[:],
                            psc[:],
                            mybir.ActivationFunctionType.Exp,
                            scale=float(SCALE),
                        )
                        for s0 in (0, 512):
                            nc.tensor.matmul(
                                po_t[:, s0 : s0 + 512],
                                v_sb[:, jg, ih, :],
                                pt_t[:, s0 : s0 + 512],
                                start=(jg == 0),
                                stop=(jg == JG - 1),
                            )
                    # normalize: partitions 0..D-1 of po_t hold the row-sum
                    # broadcast across 64 partitions (indicator cols of v_sb);
                    # partitions D..2D-1 hold O.T.
                    for s0 in (0, 512):
                        rb_t = rbpool.tile([D, 512], f32, tag="rb")
                        nc.vector.reciprocal_approx_fast(
                            rb_t[:], po_t[0:D, s0 : s0 + 512]
                        )
                        nc.vector.tensor_mul(
                            OT[:, ih, q0 + s0 : q0 + s0 + 512],
                            po_t[D : 2 * D, s0 : s0 + 512],
                            rb_t[:],
                        )

            # ---- partial projection, transposed output
            for cg in range(6):
                for n0 in range(0, N, 1024):
                    pp = pspool.tile([P, 1024], f32, tag="ps")
                    for s0 in (0, 512):
                        for ih in range(HPC):
                            nc.tensor.matmul(
                                pp[:, s0 : s0 + 512],
                                pT[:, ih, cg * P : (cg + 1) * P],
                                OT[:, ih, n0 + s0 : n0 + s0 + 512],
                                start=(ih == 0),
                                stop=(ih == HPC - 1),
                            )
                    ob = opool.tile([P, 1024], f32)
                    if (cg * 2 + n0 // 1024) % 2 == 0:
                        nc.vector.tensor_copy(ob[:], pp[:])
                    else:
                        nc.scalar.copy(ob[:], pp[:])
                    nc.sync.dma_start(out_d[:, cg, n0 : n0 + 1024], ob[:])

    nc.finalize()
    return nc


def _prep_inputs(x, mask, qkv_w, proj_w):
    """Build the 8 per-core input maps. Returns (in_maps, KP)."""
    idx = [np.nonzero(mask[b] == 0.0)[0] for b in range(B)]
    nk = max(len(i) for i in idx)
    KP = max(P, int(math.ceil(nk / P)) * P)
    JG = KP // P

    per_batch = []
    for b in range(B):
        xTb = np.ascontiguousarray(x[b].T)  # [C, N] f32
        xT_in = xTb.reshape(6, P, N).transpose(1, 0, 2).astype(BF16)
        xk = np.zeros((C, KP), np.float32)
        xk[:, : len(idx[b])] = xTb[:, idx[b]]
        xTk_in = xk.reshape(6, P, KP).transpose(1, 0, 2).astype(BF16)
        kfv = np.zeros((KP,), np.float32)
        kfv[: len(idx[b])] = 1.0
        kf_in = np.ascontiguousarray(
            np.broadcast_to(kfv.reshape(JG, P).T[:, :, None], (P, JG, D))
        ).astype(BF16)
        per_batch.append((xT_in, xTk_in, kf_in))

    in_maps = []
    for c in range(NCORES):
        b, g = c // 4, c % 4
        h0 = HPC * g
        xT_in, xTk_in, kf_in = per_batch[b]
        m = {"xT": xT_in, "xTk": xTk_in, "kf": kf_in}
        for name, off in (("wqT", 0), ("wkT", C), ("wvT", 2 * C)):
            w = qkv_w[off + h0 * D : off + (h0 + HPC) * D]  # [192, C]
            m[name] = (
                np.ascontiguousarray(w.T).reshape(6, P, 192).transpose(1, 0, 2).astype(BF16)
            )
        pT = np.stack(
            [proj_w[:, (h0 + ih) * D : (h0 + ih + 1) * D].T for ih in range(HPC)],
            axis=1,
        )  # [64, 3, 768]
        m["pT"] = pT.astype(BF16)
        in_maps.append(m)
    return in_maps, KP


_CACHE = {}


def _get_program(KP):
    if KP not in _CACHE:
        _CACHE[KP] = _build_program(KP)
    return _CACHE[KP]


def kernel(x, mask, qkv_w, proj_w, proj_b, _want_results=False):
    from concourse.bass_utils import run_bass_kernel_spmd

    x = np.asarray(x, np.float32)
    mask = np.asarray(mask, np.float32)
    qkv_w = np.asarray(qkv_w, np.float32)
    proj_w = np.asarray(proj_w, np.float32)
    proj_b = np.asarray(proj_b, np.float32)

    in_maps, KP = _prep_inputs(x, mask, qkv_w, proj_w)
    nc = _get_program(KP)
    res = run_bass_kernel_spmd(nc, in_maps, list(range(NCORES)))

    out = np.empty((B, N, C), np.float32)
    for b in range(B):
        acc = None
        for c in range(4 * b, 4 * b + 4):
            a = res.results[c]["outT"]  # [128, 6, N]
            a = np.asarray(a, np.float32).transpose(1, 0, 2).reshape(C, N)
            acc = a if acc is None else acc + a
        out[b] = acc.T + proj_b[None, :]
    if _want_results:
        return out, res
    return out


# revision 28
# speedup vs baseline: 1.4437x; 1.0259x over previous
"""Masked multi-head attention kernel for 8 Trainium2 NeuronCores.

Strategy:
  - 24 (batch, head) pairs sharded as: core c -> batch c//4, heads [3*(c%4) .. 3*(c%4)+2].
  - Key-padding mask handled by HOST-side gather: only unmasked key positions are
    shipped/computed (mask kills ~50% of keys -> scores/exp/AV work halves).
    Padded key slots get zeroed K columns (scores=0 -> exp=1) and a 0 in the
    indicator column of V, so they contribute nothing to numerator or denominator.
  - Softmax without max-subtraction (scores ~ N(0,1); masked keys excluded, exact
    match: reference's masked exp underflows to 0 in fp32).
  - Row-sum of exp folded into the AV matmul via an extra indicator column on V.
  - bf16 matmul inputs, fp32 PSUM accumulation.
  - Per-core output = transposed partial projection [768, 2048]; host sums the 4
    partials per batch, transposes, adds proj_b.
Engine split: PE matmuls; ACT only exp; DVE copies/recip/mul; DMA via sync.
"""

import math

import numpy as np
import ml_dtypes

BF16 = ml_dtypes.bfloat16
B, N, C = 2, 2048, 768
H = 12
D = 64
HPC = 3          # heads per core
P = 128
SCALE = D ** -0.5
NCORES = 8


def _build_program(KP: int):
    from concourse import bacc, mybir
    from concourse.tile import TileContext

    JG = KP // P
    f32 = mybir.dt.float32
    bf16 = mybir.dt.bfloat16
    nc = bacc.Bacc(None, target_bir_lowering=False)

    xT_d = nc.declare_dram_parameter("xT", [P, 6, N], bf16, False)
    xTk_d = nc.declare_dram_parameter("xTk", [P, 6, KP], bf16, False)
    kf_d = nc.declare_dram_parameter("kf", [P, JG, D], bf16, False)
    wq_d = nc.declare_dram_parameter("wqT", [P, 6, 192], bf16, False)
    wk_d = nc.declare_dram_parameter("wkT", [P, 6, 192], bf16, False)
    wv_d = nc.declare_dram_parameter("wvT", [P, 6, 192], bf16, False)
    pT_d = nc.declare_dram_parameter("pT", [D, HPC, C], bf16, False)
    out_d = nc.declare_dram_parameter("outT", [P, 6, N], f32, True)

    with TileContext(nc) as tc:
        with (
            tc.tile_pool(name="const", bufs=1) as cpool,
            tc.tile_pool(name="work", bufs=1) as wpool,
            tc.tile_pool(name="pt", bufs=3) as ptpool,
            tc.tile_pool(name="rb", bufs=4) as rbpool,
            tc.tile_pool(name="outp", bufs=3) as opool,
            tc.tile_pool(name="ps", bufs=2, space="PSUM") as pspool,
            tc.tile_pool(name="po", bufs=2, space="PSUM") as popool,
        ):
            # ---- load inputs
            xT = cpool.tile([P, 6, N], bf16)
            nc.sync.dma_start(xT[:], xT_d[:])
            xTk = cpool.tile([P, 6, KP], bf16)
            nc.sync.dma_start(xTk[:], xTk_d[:])
            kf = cpool.tile([P, JG, D], bf16)
            nc.sync.dma_start(kf[:], kf_d[:])
            wq = cpool.tile([P, 6, 192], bf16)
            nc.sync.dma_start(wq[:], wq_d[:])
            wk = cpool.tile([P, 6, 192], bf16)
            nc.sync.dma_start(wk[:], wk_d[:])
            wv = cpool.tile([P, 6, 192], bf16)
            nc.sync.dma_start(wv[:], wv_d[:])
            pT = cpool.tile([D, HPC, C], bf16)
            nc.sync.dma_start(pT[:], pT_d[:])

            # ---- qT / kT projections (2 heads packed per 128-row group)
            qT01 = wpool.tile([P, N], bf16)
            qT2 = wpool.tile([D, N], bf16)
            kT01 = wpool.tile([P, KP], bf16)
            kT2 = wpool.tile([D, KP], bf16)

            for (w_sb, dst_list, ncols) in (
                (wq, [(0, P, qT01), (P, D, qT2)], N),
                (wk, [(0, P, kT01), (P, D, kT2)], KP),
            ):
                for (m0, msz, dst) in dst_list:
                    for n0 in range(0, ncols, 1024):
                        nsz = min(1024, ncols - n0)
                        pq = pspool.tile([P, 1024], f32, tag="ps")
                        for s0 in range(0, nsz, 512):
                            ssz = min(512, nsz - s0)
                            for t in range(6):
                                nc.tensor.matmul(
                                    pq[:msz, s0 : s0 + ssz],
                                    w_sb[:, t, m0 : m0 + msz],
                                    (xT if ncols == N else xTk)[:, t, n0 + s0 : n0 + s0 + ssz],
                                    start=(t == 0),
                                    stop=(t == 5),
                                )
                        nc.vector.tensor_copy(dst[:msz, n0 : n0 + nsz], pq[:msz, :nsz])

            # ---- V in [key, dim] layout; cols D..2D-1 hold the keep-indicator
            # replicated 64x so the AV matmul emits the softmax denominator
            # broadcast across 64 partitions for free.
            v_sb = wpool.tile([P, JG, HPC, 2 * D], bf16)
            for jg in range(JG):
                pv = pspool.tile([P, 1024], f32, tag="ps")
                for t in range(6):
                    nc.tensor.matmul(
                        pv[:, :192],
                        xTk[:, t, jg * P : (jg + 1) * P],
                        wv[:, t, :],
                        start=(t == 0),
                        stop=(t == 5),
                    )
                for ih in range(HPC):
                    nc.vector.tensor_copy(v_sb[:, jg, ih, 0:D], kf[:, jg, :])
                    nc.vector.tensor_copy(
                        v_sb[:, jg, ih, D : 2 * D], pv[:, ih * D : (ih + 1) * D]
                    )

            # ---- attention per half-head (1024 queries): two halves in flight
            # (po bufs=2) so PE always has the other half's scores to issue
            # while ScalarE's exp lags.
            NH = N // 2
            OT = [
                wpool.tile([D, HPC, NH], bf16, name="OTa"),
                wpool.tile([D, HPC, NH], bf16, name="OTb"),
            ]
            for ih in range(HPC):
                qT_h = qT01[D * ih : D * (ih + 1), :] if ih < 2 else qT2[:, :]
                kT_h = kT01[D * ih : D * (ih + 1), :] if ih < 2 else kT2[:, :]
                for nh in range(2):
                    q0 = nh * NH
                    po_t = popool.tile([P, NH], f32)
                    for jg in range(JG):
                        pt_t = ptpool.tile([P, NH], bf16)
                        psc = pspool.tile([P, 1024], f32, tag="ps")
                        for s0 in (0, 512):
                            nc.tensor.matmul(
                                psc[:, s0 : s0 + 512],
                                kT_h[:, jg * P : (jg + 1) * P],
                                qT_h[:, q0 + s0 : q0 + s0 + 512],
                                start=True,
                                stop=True,
                            )
                        nc.scalar.activation(
                            pt_t[:],
                            psc[:],
                            mybir.ActivationFunctionType.Exp,
                            scale=float(SCALE),
                        )
                        for s0 in (0, 512):
                            nc.tensor.matmul(
                                po_t[:, s0 : s0 + 512],
                                v_sb[:, jg, ih, :],
                                pt_t[:, s0 : s0 + 512],
                                start=(jg == 0),
                                stop=(jg == JG - 1),
                            )
                    # normalize: partitions 0..D-1 of po_t hold the row-sum
                    # broadcast across 64 partitions (indicator cols of v_sb);
                    # partitions D..2D-1 hold O.T.
                    for s0 in (0, 512):
                        rb_t = rbpool.tile([D, 512], f32, tag="rb")
                        nc.vector.reciprocal_approx_fast(
                            rb_t[:], po_t[0:D, s0 : s0 + 512]
                        )
                        nc.vector.tensor_mul(
                            OT[nh][:, ih, s0 : s0 + 512],
                            po_t[D : 2 * D, s0 : s0 + 512],
                            rb_t[:],
                        )

            # ---- partial projection, transposed output. Query-half outer so the
            # first half's tiles (dep: OT[0] only) overlap the last half-head.
            for nh in range(2):
                for cg in range(6):
                    pp = pspool.tile([P, 1024], f32, tag="ps")
                    for s0 in (0, 512):
                        for ih in range(HPC):
                            nc.tensor.matmul(
                                pp[:, s0 : s0 + 512],
                                pT[:, ih, cg * P : (cg + 1) * P],
                                OT[nh][:, ih, s0 : s0 + 512],
                                start=(ih == 0),
                                stop=(ih == HPC - 1),
                            )
                    ob = opool.tile([P, 1024], f32)
                    if cg % 2 == 0:
                        nc.vector.tensor_copy(ob[:], pp[:])
                    else:
                        nc.scalar.copy(ob[:], pp[:])
                    nc.sync.dma_start(out_d[:, cg, nh * NH : nh * NH + 1024], ob[:])

    nc.finalize()
    return nc


def _prep_inputs(x, mask, qkv_w, proj_w):
    """Build the 8 per-core input maps. Returns (in_maps, KP)."""
    idx = [np.nonzero(mask[b] == 0.0)[0] for b in range(B)]
    nk = max(len(i) for i in idx)
    KP = max(P, int(math.ceil(nk / P)) * P)
    JG = KP // P

    per_batch = []
    for b in range(B):
        xTb = np.ascontiguousarray(x[b].T)  # [C, N] f32
        xT_in = xTb.reshape(6, P, N).transpose(1, 0, 2).astype(BF16)
        xk = np.zeros((C, KP), np.float32)
        xk[:, : len(idx[b])] = xTb[:, idx[b]]
        xTk_in = xk.reshape(6, P, KP).transpose(1, 0, 2).astype(BF16)
        kfv = np.zeros((KP,), np.float32)
        kfv[: len(idx[b])] = 1.0
        kf_in = np.ascontiguousarray(
            np.broadcast_to(kfv.reshape(JG, P).T[:, :, None], (P, JG, D))
        ).astype(BF16)
        per_batch.append((xT_in, xTk_in, kf_in))

    in_maps = []
    for c in range(NCORES):
        b, g = c // 4, c % 4
        h0 = HPC * g
        xT_in, xTk_in, kf_in = per_batch[b]
        m = {"xT": xT_in, "xTk": xTk_in, "kf": kf_in}
        for name, off in (("wqT", 0), ("wkT", C), ("wvT", 2 * C)):
            w = qkv_w[off + h0 * D : off + (h0 + HPC) * D]  # [192, C]
            m[name] = (
                np.ascontiguousarray(w.T).reshape(6, P, 192).transpose(1, 0, 2).astype(BF16)
            )
        pT = np.stack(
            [proj_w[:, (h0 + ih) * D : (h0 + ih + 1) * D].T for ih in range(HPC)],
            axis=1,
        )  # [64, 3, 768]
        m["pT"] = pT.astype(BF16)
        in_maps.append(m)
    return in_maps, KP


_CACHE = {}


def _get_program(KP):
    if KP not in _CACHE:
        _CACHE[KP] = _build_program(KP)
    return _CACHE[KP]


def kernel(x, mask, qkv_w, proj_w, proj_b, _want_results=False):
    from concourse.bass_utils import run_bass_kernel_spmd

    x = np.asarray(x, np.float32)
    mask = np.asarray(mask, np.float32)
    qkv_w = np.asarray(qkv_w, np.float32)
    proj_w = np.asarray(proj_w, np.float32)
    proj_b = np.asarray(proj_b, np.float32)

    in_maps, KP = _prep_inputs(x, mask, qkv_w, proj_w)
    nc = _get_program(KP)
    res = run_bass_kernel_spmd(nc, in_maps, list(range(NCORES)))

    out = np.empty((B, N, C), np.float32)
    for b in range(B):
        acc = None
        for c in range(4 * b, 4 * b + 4):
            a = res.results[c]["outT"]  # [128, 6, N]
            a = np.asarray(a, np.float32).transpose(1, 0, 2).reshape(C, N)
            acc = a if acc is None else acc + a
        out[b] = acc.T + proj_b[None, :]
    if _want_results:
        return out, res
    return out


# revision 29
# speedup vs baseline: 1.4787x; 1.0243x over previous
"""Masked multi-head attention kernel for 8 Trainium2 NeuronCores.

Strategy:
  - 24 (batch, head) pairs sharded as: core c -> batch c//4, heads [3*(c%4) .. 3*(c%4)+2].
  - Key-padding mask handled by HOST-side gather: only unmasked key positions are
    shipped/computed (mask kills ~50% of keys -> scores/exp/AV work halves).
    Padded key slots get zeroed K columns (scores=0 -> exp=1) and a 0 in the
    indicator column of V, so they contribute nothing to numerator or denominator.
  - Softmax without max-subtraction (scores ~ N(0,1); masked keys excluded, exact
    match: reference's masked exp underflows to 0 in fp32).
  - Row-sum of exp folded into the AV matmul via an extra indicator column on V.
  - bf16 matmul inputs, fp32 PSUM accumulation.
  - Per-core output = transposed partial projection [768, 2048]; host sums the 4
    partials per batch, transposes, adds proj_b.
Engine split: PE matmuls; ACT only exp; DVE copies/recip/mul; DMA via sync.
"""

import math

import numpy as np
import ml_dtypes

BF16 = ml_dtypes.bfloat16
B, N, C = 2, 2048, 768
H = 12
D = 64
HPC = 3          # heads per core
P = 128
SCALE = D ** -0.5
NCORES = 8


def _build_program(KP: int):
    from concourse import bacc, mybir
    from concourse.tile import TileContext

    JG = KP // P
    f32 = mybir.dt.float32
    bf16 = mybir.dt.bfloat16
    nc = bacc.Bacc(None, target_bir_lowering=False)

    xT_d = nc.declare_dram_parameter("xT", [P, 6, N], bf16, False)
    xTk_d = nc.declare_dram_parameter("xTk", [P, 6, KP], bf16, False)
    kf_d = nc.declare_dram_parameter("kf", [P, JG, D], bf16, False)
    wq_d = nc.declare_dram_parameter("wqT", [P, 6, 192], bf16, False)
    wk_d = nc.declare_dram_parameter("wkT", [P, 6, 192], bf16, False)
    wv_d = nc.declare_dram_parameter("wvT", [P, 6, 192], bf16, False)
    pT_d = nc.declare_dram_parameter("pT", [D, HPC, C], bf16, False)
    out_d = nc.declare_dram_parameter("outT", [P, 6, N], f32, True)

    with TileContext(nc) as tc:
        with (
            tc.tile_pool(name="const", bufs=1) as cpool,
            tc.tile_pool(name="work", bufs=1) as wpool,
            tc.tile_pool(name="pt", bufs=3) as ptpool,
            tc.tile_pool(name="rb", bufs=4) as rbpool,
            tc.tile_pool(name="outp", bufs=3) as opool,
            tc.tile_pool(name="ps", bufs=2, space="PSUM") as pspool,
            tc.tile_pool(name="po", bufs=2, space="PSUM") as popool,
        ):
            # ---- load inputs: weights first, x chunked per contraction tile so
            # the first matmuls start after ~1/6 of the x transfer.
            wq = cpool.tile([P, 6, 192], bf16)
            nc.sync.dma_start(wq[:], wq_d[:])
            wk = cpool.tile([P, 6, 192], bf16)
            nc.sync.dma_start(wk[:], wk_d[:])
            wv = cpool.tile([P, 6, 192], bf16)
            nc.sync.dma_start(wv[:], wv_d[:])
            xT = cpool.tile([P, 6, N], bf16)
            xTk = cpool.tile([P, 6, KP], bf16)
            for t in range(6):
                nc.sync.dma_start(xT[:, t, :], xT_d[:, t, :])
                nc.sync.dma_start(xTk[:, t, :], xTk_d[:, t, :])
            kf = cpool.tile([P, JG, D], bf16)
            nc.sync.dma_start(kf[:], kf_d[:])
            pT = cpool.tile([D, HPC, C], bf16)
            nc.sync.dma_start(pT[:], pT_d[:])

            # ---- qT / kT projections (2 heads packed per 128-row group)
            qT01 = wpool.tile([P, N], bf16)
            qT2 = wpool.tile([D, N], bf16)
            kT01 = wpool.tile([P, KP], bf16)
            kT2 = wpool.tile([D, KP], bf16)

            for (w_sb, dst_list, ncols) in (
                (wq, [(0, P, qT01), (P, D, qT2)], N),
                (wk, [(0, P, kT01), (P, D, kT2)], KP),
            ):
                for (m0, msz, dst) in dst_list:
                    for n0 in range(0, ncols, 1024):
                        nsz = min(1024, ncols - n0)
                        pq = pspool.tile([P, 1024], f32, tag="ps")
                        for s0 in range(0, nsz, 512):
                            ssz = min(512, nsz - s0)
                            for t in range(6):
                                nc.tensor.matmul(
                                    pq[:msz, s0 : s0 + ssz],
                                    w_sb[:, t, m0 : m0 + msz],
                                    (xT if ncols == N else xTk)[:, t, n0 + s0 : n0 + s0 + ssz],
                                    start=(t == 0),
                                    stop=(t == 5),
                                )
                        nc.vector.tensor_copy(dst[:msz, n0 : n0 + nsz], pq[:msz, :nsz])

            # ---- V in [key, dim] layout; cols D..2D-1 hold the keep-indicator
            # replicated 64x so the AV matmul emits the softmax denominator
            # broadcast across 64 partitions for free.
            v_sb = wpool.tile([P, JG, HPC, 2 * D], bf16)
            for jg in range(JG):
                pv = pspool.tile([P, 1024], f32, tag="ps")
                for t in range(6):
                    nc.tensor.matmul(
                        pv[:, :192],
                        xTk[:, t, jg * P : (jg + 1) * P],
                        wv[:, t, :],
                        start=(t == 0),
                        stop=(t == 5),
                    )
                for ih in range(HPC):
                    nc.vector.tensor_copy(v_sb[:, jg, ih, 0:D], kf[:, jg, :])
                    nc.vector.tensor_copy(
                        v_sb[:, jg, ih, D : 2 * D], pv[:, ih * D : (ih + 1) * D]
                    )

            # ---- attention per half-head (1024 queries): two halves in flight
            # (po bufs=2) so PE always has the other half's scores to issue
            # while ScalarE's exp lags.
            NH = N // 2
            OT = [
                wpool.tile([D, HPC, NH], bf16, name="OTa"),
                wpool.tile([D, HPC, NH], bf16, name="OTb"),
            ]
            for ih in range(HPC):
                qT_h = qT01[D * ih : D * (ih + 1), :] if ih < 2 else qT2[:, :]
                kT_h = kT01[D * ih : D * (ih + 1), :] if ih < 2 else kT2[:, :]
                for nh in range(2):
                    q0 = nh * NH
                    po_t = popool.tile([P, NH], f32)
                    for jg in range(JG):
                        pt_t = ptpool.tile([P, NH], bf16)
                        psc = pspool.tile([P, 1024], f32, tag="ps")
                        for s0 in (0, 512):
                            nc.tensor.matmul(
                                psc[:, s0 : s0 + 512],
                                kT_h[:, jg * P : (jg + 1) * P],
                                qT_h[:, q0 + s0 : q0 + s0 + 512],
                                start=True,
                                stop=True,
                            )
                        nc.scalar.activation(
                            pt_t[:],
                            psc[:],
                            mybir.ActivationFunctionType.Exp,
                            scale=float(SCALE),
                        )
                        for s0 in (0, 512):
                            nc.tensor.matmul(
                                po_t[:, s0 : s0 + 512],
                                v_sb[:, jg, ih, :],
                                pt_t[:, s0 : s0 + 512],
                                start=(jg == 0),
                                stop=(jg == JG - 1),
                            )
                    # normalize: partitions 0..D-1 of po_t hold the row-sum
                    # broadcast across 64 partitions (indicator cols of v_sb);
                    # partitions D..2D-1 hold O.T.
                    for s0 in (0, 512):
                        rb_t = rbpool.tile([D, 512], f32, tag="rb")
                        nc.vector.reciprocal_approx_fast(
                            rb_t[:], po_t[0:D, s0 : s0 + 512]
                        )
                        nc.vector.tensor_mul(
                            OT[nh][:, ih, s0 : s0 + 512],
                            po_t[D : 2 * D, s0 : s0 + 512],
                            rb_t[:],
                        )

            # ---- partial projection, transposed output. Query-half outer so the
            # first half's tiles (dep: OT[0] only) overlap the last half-head.
            for nh in range(2):
                for cg in range(6):
                    pp = pspool.tile([P, 1024], f32, tag="ps")
                    for s0 in (0, 512):
                        for ih in range(HPC):
                            nc.tensor.matmul(
                                pp[:, s0 : s0 + 512],
                                pT[:, ih, cg * P : (cg + 1) * P],
                                OT[nh][:, ih, s0 : s0 + 512],
                                start=(ih == 0),
                                stop=(ih == HPC - 1),
                            )
                    ob = opool.tile([P, 1024], f32)
                    if cg % 2 == 0:
                        nc.vector.tensor_copy(ob[:], pp[:])
                    else:
                        nc.scalar.copy(ob[:], pp[:])
                    nc.sync.dma_start(out_d[:, cg, nh * NH : nh * NH + 1024], ob[:])

    nc.finalize()
    return nc


def _prep_inputs(x, mask, qkv_w, proj_w):
    """Build the 8 per-core input maps. Returns (in_maps, KP)."""
    idx = [np.nonzero(mask[b] == 0.0)[0] for b in range(B)]
    nk = max(len(i) for i in idx)
    KP = max(P, int(math.ceil(nk / P)) * P)
    JG = KP // P

    per_batch = []
    for b in range(B):
        xTb = np.ascontiguousarray(x[b].T)  # [C, N] f32
        xT_in = xTb.reshape(6, P, N).transpose(1, 0, 2).astype(BF16)
        xk = np.zeros((C, KP), np.float32)
        xk[:, : len(idx[b])] = xTb[:, idx[b]]
        xTk_in = xk.reshape(6, P, KP).transpose(1, 0, 2).astype(BF16)
        kfv = np.zeros((KP,), np.float32)
        kfv[: len(idx[b])] = 1.0
        kf_in = np.ascontiguousarray(
            np.broadcast_to(kfv.reshape(JG, P).T[:, :, None], (P, JG, D))
        ).astype(BF16)
        per_batch.append((xT_in, xTk_in, kf_in))

    in_maps = []
    for c in range(NCORES):
        b, g = c // 4, c % 4
        h0 = HPC * g
        xT_in, xTk_in, kf_in = per_batch[b]
        m = {"xT": xT_in, "xTk": xTk_in, "kf": kf_in}
        for name, off in (("wqT", 0), ("wkT", C), ("wvT", 2 * C)):
            w = qkv_w[off + h0 * D : off + (h0 + HPC) * D]  # [192, C]
            m[name] = (
                np.ascontiguousarray(w.T).reshape(6, P, 192).transpose(1, 0, 2).astype(BF16)
            )
        pT = np.stack(
            [proj_w[:, (h0 + ih) * D : (h0 + ih + 1) * D].T for ih in range(HPC)],
            axis=1,
        )  # [64, 3, 768]
        m["pT"] = pT.astype(BF16)
        in_maps.append(m)
    return in_maps, KP


_CACHE = {}


def _get_program(KP):
    if KP not in _CACHE:
        _CACHE[KP] = _build_program(KP)
    return _CACHE[KP]


def kernel(x, mask, qkv_w, proj_w, proj_b, _want_results=False):
    from concourse.bass_utils import run_bass_kernel_spmd

    x = np.asarray(x, np.float32)
    mask = np.asarray(mask, np.float32)
    qkv_w = np.asarray(qkv_w, np.float32)
    proj_w = np.asarray(proj_w, np.float32)
    proj_b = np.asarray(proj_b, np.float32)

    in_maps, KP = _prep_inputs(x, mask, qkv_w, proj_w)
    nc = _get_program(KP)
    res = run_bass_kernel_spmd(nc, in_maps, list(range(NCORES)))

    out = np.empty((B, N, C), np.float32)
    for b in range(B):
        acc = None
        for c in range(4 * b, 4 * b + 4):
            a = res.results[c]["outT"]  # [128, 6, N]
            a = np.asarray(a, np.float32).transpose(1, 0, 2).reshape(C, N)
            acc = a if acc is None else acc + a
        out[b] = acc.T + proj_b[None, :]
    if _want_results:
        return out, res
    return out
